# revision 27
# baseline (speedup 1.0000x reference)
"""Trainium2 Bass kernel for nn_Attention_20925080666453.

Computation (faithful to the torch module quirk):
    e = (Q @ K) / sqrt(512)            # [B,H,S,S]
    a = softmax(e, axis=1)             # softmax over the HEAD axis
    o = a @ V                          # [B,H,S,d]
    out = o.reshape(B, S, H*d)

Sharding: 8 cores = batch (2) x query-chunk (4). The head-axis softmax couples
all 8 heads for a fixed (b, s, t), so every core keeps all heads for its query
chunk; no collectives are needed. K and V for the core's batch are duplicated
across the 4 query-chunk cores.

Per-core layout (b fixed, s_chunk of 1024 queries):
  - scores computed transposed: e_T[t, s] with lhsT = K[d, t-tile] (stationary)
    and rhs = Q^T[d, s-block] (streaming), PSUM [t=128, s=512], one bank/head.
  - exp on ScalarE directly from PSUM (scale fused), bf16 out to SBUF.
  - softmax over heads: tree-sum + reciprocal + scale on VectorE (bf16).
  - o_T[d, s] += V[t-tile, d].T @ a_T[t-tile, s-block] accumulated over the
    32 t-tiles in PSUM; two heads share one PSUM bank via column tiling
    (head pair packs d=64+64 into 128 partitions).
  - output stored as o_T [head-pair, 2*64, s]; host reassembles to [B,S,512].
"""

import os
import sys
import threading

sys.path.insert(0, "/opt/trn_rl_repo")

import ml_dtypes
import numpy as np

import concourse.bacc as bacc
import concourse.bass as bass
import concourse.mybir as mybir
import concourse.tile as tile
from concourse.bass_utils import run_bass_kernel_spmd

# Problem dims
B, H, S, D = 2, 8, 4096, 64
HIDDEN = H * D
SCALE = float(1.0 / np.sqrt(np.float32(HIDDEN)))

P = 128              # partitions
NPAIR = H // 2       # head pairs
N_CORES = 8
S_CHUNKS = 4         # query chunks per batch
S_LOC = S // S_CHUNKS    # 1024 queries per core
SBLK = 512               # s-block (one PSUM bank of fp32)
NSB = S_LOC // SBLK      # 2
NTT = S // P             # 32 key tiles of 128

_cache = {"nc": None}
_lock = threading.Lock()


def _build():
    nc = bacc.Bacc(
        "TRN2",
        target_bir_lowering=False,
        debug=False,
        enable_asserts=True,
        num_devices=N_CORES,
    )
    f32 = mybir.dt.float32
    bf16 = mybir.dt.bfloat16
    f16 = mybir.dt.float16

    qt_d = nc.dram_tensor("QT", [H, D, S_LOC], bf16, kind="ExternalInput").ap()
    k_d = nc.dram_tensor("K", [H, D, S], bf16, kind="ExternalInput").ap()
    v_d = nc.dram_tensor("V", [H, P, NTT, D], f16, kind="ExternalInput").ap()
    out_d = nc.dram_tensor("OUT", [NPAIR, P, S_LOC], f32, kind="ExternalOutput").ap()

    Exp = mybir.ActivationFunctionType.Exp

    with tile.TileContext(nc) as tc:
        with (
            tc.tile_pool(name="consts", bufs=1) as consts,
            tc.tile_pool(name="score", bufs=2, space="PSUM") as score_pool,
            tc.tile_pool(name="oaccp", bufs=1, space="PSUM") as oacc_pool,
            tc.tile_pool(name="expp", bufs=4) as exp_pool,
            tc.tile_pool(name="apool", bufs=4) as a_pool,
            tc.tile_pool(name="tmp", bufs=4) as tmp_pool,
            tc.tile_pool(name="outp", bufs=8) as outp,
        ):
            # resident inputs
            k_sb = [None] * NPAIR
            qt_sb = [None] * NPAIR
            v_sb = [None] * H

            def load_kqt(pr):
                kt = consts.tile([P, S], bf16, name=f"k_sb{pr}")
                nc.sync.dma_start(
                    out=kt, in_=k_d[2 * pr : 2 * pr + 2].rearrange("h d t -> (h d) t")
                )
                k_sb[pr] = kt
                qt = consts.tile([P, S_LOC], bf16, name=f"qt_sb{pr}")
                nc.sync.dma_start(
                    out=qt, in_=qt_d[2 * pr : 2 * pr + 2].rearrange("h d s -> (h d) s")
                )
                qt_sb[pr] = qt

            for pr in range(NPAIR):
                load_kqt(pr)
            for h in range(H):
                vt = consts.tile([P, NTT, D], f16, name=f"v_sb{h}")
                nc.sync.dma_start(out=vt, in_=v_d[h])
                v_sb[h] = vt

            for sb in range(NSB):
                oacc = [
                    oacc_pool.tile([P, SBLK], f32, name=f"oacc{pr}")
                    for pr in range(NPAIR)
                ]
                for tt in range(NTT):
                    exp_sb = exp_pool.tile([P, H, SBLK], f16, name="exp_sb")
                    for pr in range(NPAIR):
                        ps = score_pool.tile([P, 2, SBLK], f32, name="score")
                        for j in range(2):
                            # e_T[t, s] for head 2*pr+j; j=1 uses PE rows 64-127
                            nc.tensor.matmul(
                                out=ps[:, j, :],
                                lhsT=k_sb[pr][j * D : (j + 1) * D, tt * P : (tt + 1) * P],
                                rhs=qt_sb[pr][
                                    j * D : (j + 1) * D, sb * SBLK : (sb + 1) * SBLK
                                ],
                                start=True,
                                stop=True,
                            )
                        nc.scalar.activation(
                            out=exp_sb[:, 2 * pr : 2 * pr + 2, :],
                            in_=ps[:, :, :],
                            func=Exp,
                            scale=SCALE,
                        )
                    # softmax over heads: Z = sum_h exp_h ; a_h = exp_h / Z
                    z4 = tmp_pool.tile([P, 4, SBLK], f16, name="z4")
                    nc.vector.tensor_add(z4, exp_sb[:, 0:4, :], exp_sb[:, 4:8, :])
                    z2 = tmp_pool.tile([P, 2, SBLK], f16, name="z2")
                    nc.vector.tensor_add(z2, z4[:, 0:2, :], z4[:, 2:4, :])
                    z1 = tmp_pool.tile([P, SBLK], f16, name="z1")
                    nc.vector.tensor_add(z1, z2[:, 0, :], z2[:, 1, :])
                    # custom-DVE recip; bf16 input is converted to f32 by the
                    # read stage before the fp32 bit-trick seed
                    from concourse.dve_ops import (
                        RECIP_APPROX_FAST_CONSTS as _RC,
                        RECIPROCAL_APPROX_FAST as _RAF,
                    )
                    rf = tmp_pool.tile([P, SBLK], f32, name="rf")
                    nc.vector._custom_dve(
                        _RAF, out=rf, in0=z1, s0=_RC["s0"], s1=_RC["s1"],
                        imm2=_RC["imm2"],
                    )
                    rcp = tmp_pool.tile([P, SBLK], f16, name="rcp")
                    nc.scalar.copy(out=rcp, in_=rf)
                    a_sb = a_pool.tile([P, H, SBLK], f16, name="a_sb")
                    # single mul with r broadcast over the head axis (step-0 dim)
                    rcp_b = bass.AP(
                        tensor=rcp.tensor,
                        offset=rcp.offset,
                        ap=[rcp.ap[0], [0, H], rcp.ap[1]],
                    )
                    nc.vector.tensor_mul(a_sb, exp_sb, rcp_b)
                    for pr in range(NPAIR):
                        for j in range(2):
                            h = 2 * pr + j
                            # o_T[d, s] accumulation; j=1 targets PSUM partitions 64-127
                            nc.tensor.matmul(
                                out=oacc[pr][j * D : (j + 1) * D, :],
                                lhsT=v_sb[h][:, tt, :],
                                rhs=a_sb[:, h, :],
                                start=(tt == 0),
                                stop=(tt == NTT - 1),
                            )
                for pr in range(NPAIR):
                    ot = outp.tile([P, SBLK], f32, name="ot")
                    nc.scalar.copy(out=ot, in_=oacc[pr])
                    nc.sync.dma_start(
                        out=out_d[pr, :, sb * SBLK : (sb + 1) * SBLK], in_=ot
                    )

    nc.compile()
    return nc


def _get_nc():
    with _lock:
        if _cache["nc"] is None:
            _cache["nc"] = _build()
        return _cache["nc"]


def _prep_inputs(Q, K, V):
    Q = np.asarray(Q, dtype=np.float32)
    K = np.asarray(K, dtype=np.float32)
    V = np.asarray(V, dtype=np.float32)
    # Q^T per head: [B, H, D, S], bf16 for full-rate PE streaming
    qt = np.ascontiguousarray(Q.transpose(0, 1, 3, 2)).astype(ml_dtypes.bfloat16)
    kb = K.astype(ml_dtypes.bfloat16)
    # V pre-swizzled to SBUF layout [B, H, p, t_tile, d], bf16
    vp = np.ascontiguousarray(
        V.reshape(B, H, NTT, P, D).transpose(0, 1, 3, 2, 4)
    ).astype(np.float16)
    in_maps = []
    for c in range(N_CORES):
        b, sc = divmod(c, S_CHUNKS)
        in_maps.append(
            {
                "QT": np.ascontiguousarray(
                    qt[b, :, :, sc * S_LOC : (sc + 1) * S_LOC]
                ),
                "K": np.ascontiguousarray(kb[b]),
                "V": vp[b],
            }
        )
    return in_maps


def _assemble(results):
    # The reference output is a RAW reshape of contiguous [B, H, S, d] to
    # [B, S, H*d] (torch .view quirk), NOT a head-transpose. So build
    # o[B, H, S, d] and reshape.
    o_full = np.empty((B, H, S, D), dtype=np.float32)
    for c in range(N_CORES):
        b, sc = divmod(c, S_CHUNKS)
        shard = results[c]["OUT"]  # [NPAIR, 128, S_LOC] = [pair, (half d), s]
        o_full[b, :, sc * S_LOC : (sc + 1) * S_LOC, :] = (
            shard.reshape(NPAIR, 2, D, S_LOC).transpose(0, 1, 3, 2).reshape(
                H, S_LOC, D
            )
        )
    return o_full.reshape(B, S, HIDDEN)


def run(Q, K, V, trace=False, **run_kwargs):
    nc = _get_nc()
    in_maps = _prep_inputs(Q, K, V)
    res = run_bass_kernel_spmd(
        nc, in_maps, core_ids=list(range(N_CORES)), trace=trace, **run_kwargs
    )
    return _assemble(res.results), res


def kernel(Q, K, V):
    # Force the no-trace path: the NTFF profile hook is not wired up in a
    # bare environment, and BASS_TRACE in the ambient env would crash.
    prev = os.environ.get("BASS_NEVER_TRACE")
    os.environ["BASS_NEVER_TRACE"] = "1"
    try:
        out, _ = run(Q, K, V, trace=False)
    finally:
        if prev is None:
            os.environ.pop("BASS_NEVER_TRACE", None)
        else:
            os.environ["BASS_NEVER_TRACE"] = prev
    return out


# revision 28
# speedup vs baseline: 1.0067x; 1.0067x over previous
"""Trainium2 Bass kernel for nn_Attention_20925080666453.

Computation (faithful to the torch module quirk):
    e = (Q @ K) / sqrt(512)            # [B,H,S,S]
    a = softmax(e, axis=1)             # softmax over the HEAD axis
    o = a @ V                          # [B,H,S,d]
    out = o.reshape(B, S, H*d)

Sharding: 8 cores = batch (2) x query-chunk (4). The head-axis softmax couples
all 8 heads for a fixed (b, s, t), so every core keeps all heads for its query
chunk; no collectives are needed. K and V for the core's batch are duplicated
across the 4 query-chunk cores.

Per-core layout (b fixed, s_chunk of 1024 queries):
  - scores computed transposed: e_T[t, s] with lhsT = K[d, t-tile] (stationary)
    and rhs = Q^T[d, s-block] (streaming), PSUM [t=128, s=512], one bank/head.
  - exp on ScalarE directly from PSUM (scale fused), bf16 out to SBUF.
  - softmax over heads: tree-sum + reciprocal + scale on VectorE (bf16).
  - o_T[d, s] += V[t-tile, d].T @ a_T[t-tile, s-block] accumulated over the
    32 t-tiles in PSUM; two heads share one PSUM bank via column tiling
    (head pair packs d=64+64 into 128 partitions).
  - output stored as o_T [head-pair, 2*64, s]; host reassembles to [B,S,512].
"""

import os
import sys
import threading

sys.path.insert(0, "/opt/trn_rl_repo")

import ml_dtypes
import numpy as np

import concourse.bacc as bacc
import concourse.bass as bass
import concourse.mybir as mybir
import concourse.tile as tile
from concourse.bass_utils import run_bass_kernel_spmd

# Problem dims
B, H, S, D = 2, 8, 4096, 64
HIDDEN = H * D
SCALE = float(1.0 / np.sqrt(np.float32(HIDDEN)))

P = 128              # partitions
NPAIR = H // 2       # head pairs
N_CORES = 8
S_CHUNKS = 4         # query chunks per batch
S_LOC = S // S_CHUNKS    # 1024 queries per core
SBLK = 512               # s-block (one PSUM bank of fp32)
NSB = S_LOC // SBLK      # 2
NTT = S // P             # 32 key tiles of 128

_cache = {"nc": None}
_lock = threading.Lock()


def _build():
    nc = bacc.Bacc(
        "TRN2",
        target_bir_lowering=False,
        debug=False,
        enable_asserts=True,
        num_devices=N_CORES,
    )
    f32 = mybir.dt.float32
    bf16 = mybir.dt.bfloat16
    f16 = mybir.dt.float16

    qt_d = nc.dram_tensor("QT", [H, D, S_LOC], f16, kind="ExternalInput").ap()
    k_d = nc.dram_tensor("K", [H, D, S], f16, kind="ExternalInput").ap()
    v_d = nc.dram_tensor("V", [H, P, NTT, D], f16, kind="ExternalInput").ap()
    out_d = nc.dram_tensor("OUT", [NPAIR, P, S_LOC], f32, kind="ExternalOutput").ap()

    Exp = mybir.ActivationFunctionType.Exp

    with tile.TileContext(nc) as tc:
        with (
            tc.tile_pool(name="consts", bufs=1) as consts,
            tc.tile_pool(name="score", bufs=2, space="PSUM") as score_pool,
            tc.tile_pool(name="oaccp", bufs=1, space="PSUM") as oacc_pool,
            tc.tile_pool(name="expp", bufs=4) as exp_pool,
            tc.tile_pool(name="apool", bufs=4) as a_pool,
            tc.tile_pool(name="tmp", bufs=4) as tmp_pool,
            tc.tile_pool(name="outp", bufs=8) as outp,
        ):
            # resident inputs
            k_sb = [None] * NPAIR
            qt_sb = [None] * NPAIR
            v_sb = [None] * H

            def load_kqt(pr):
                kt = consts.tile([P, S], f16, name=f"k_sb{pr}")
                nc.sync.dma_start(
                    out=kt, in_=k_d[2 * pr : 2 * pr + 2].rearrange("h d t -> (h d) t")
                )
                k_sb[pr] = kt
                qt = consts.tile([P, S_LOC], f16, name=f"qt_sb{pr}")
                nc.sync.dma_start(
                    out=qt, in_=qt_d[2 * pr : 2 * pr + 2].rearrange("h d s -> (h d) s")
                )
                qt_sb[pr] = qt

            for pr in range(NPAIR):
                load_kqt(pr)
            for h in range(H):
                vt = consts.tile([P, NTT, D], f16, name=f"v_sb{h}")
                nc.sync.dma_start(out=vt, in_=v_d[h])
                v_sb[h] = vt

            for sb in range(NSB):
                oacc = [
                    oacc_pool.tile([P, SBLK], f32, name=f"oacc{pr}")
                    for pr in range(NPAIR)
                ]
                for tt in range(NTT):
                    exp_sb = exp_pool.tile([P, H, SBLK], f16, name="exp_sb")
                    for pr in range(NPAIR):
                        ps = score_pool.tile([P, 2, SBLK], f32, name="score")
                        for j in range(2):
                            # e_T[t, s] for head 2*pr+j; j=1 uses PE rows 64-127
                            nc.tensor.matmul(
                                out=ps[:, j, :],
                                lhsT=k_sb[pr][j * D : (j + 1) * D, tt * P : (tt + 1) * P],
                                rhs=qt_sb[pr][
                                    j * D : (j + 1) * D, sb * SBLK : (sb + 1) * SBLK
                                ],
                                start=True,
                                stop=True,
                            )
                        nc.scalar.activation(
                            out=exp_sb[:, 2 * pr : 2 * pr + 2, :],
                            in_=ps[:, :, :],
                            func=Exp,
                            scale=SCALE,
                        )
                    # softmax over heads: Z = sum_h exp_h ; a_h = exp_h / Z
                    z4 = tmp_pool.tile([P, 4, SBLK], f16, name="z4")
                    nc.vector.tensor_add(z4, exp_sb[:, 0:4, :], exp_sb[:, 4:8, :])
                    z2 = tmp_pool.tile([P, 2, SBLK], f16, name="z2")
                    nc.vector.tensor_add(z2, z4[:, 0:2, :], z4[:, 2:4, :])
                    z1 = tmp_pool.tile([P, SBLK], f16, name="z1")
                    nc.vector.tensor_add(z1, z2[:, 0, :], z2[:, 1, :])
                    # custom-DVE recip; bf16 input is converted to f32 by the
                    # read stage before the fp32 bit-trick seed
                    from concourse.dve_ops import (
                        RECIP_APPROX_FAST_CONSTS as _RC,
                        RECIPROCAL_APPROX_FAST as _RAF,
                    )
                    rf = tmp_pool.tile([P, SBLK], f32, name="rf")
                    nc.vector._custom_dve(
                        _RAF, out=rf, in0=z1, s0=_RC["s0"], s1=_RC["s1"],
                        imm2=_RC["imm2"],
                    )
                    rcp = tmp_pool.tile([P, SBLK], f16, name="rcp")
                    nc.scalar.copy(out=rcp, in_=rf)
                    a_sb = a_pool.tile([P, H, SBLK], f16, name="a_sb")
                    # single mul with r broadcast over the head axis (step-0 dim)
                    rcp_b = bass.AP(
                        tensor=rcp.tensor,
                        offset=rcp.offset,
                        ap=[rcp.ap[0], [0, H], rcp.ap[1]],
                    )
                    nc.vector.tensor_mul(a_sb, exp_sb, rcp_b)
                    for pr in range(NPAIR):
                        for j in range(2):
                            h = 2 * pr + j
                            # o_T[d, s] accumulation; j=1 targets PSUM partitions 64-127
                            nc.tensor.matmul(
                                out=oacc[pr][j * D : (j + 1) * D, :],
                                lhsT=v_sb[h][:, tt, :],
                                rhs=a_sb[:, h, :],
                                start=(tt == 0),
                                stop=(tt == NTT - 1),
                            )
                for pr in range(NPAIR):
                    ot = outp.tile([P, SBLK], f32, name="ot")
                    nc.scalar.copy(out=ot, in_=oacc[pr])
                    nc.sync.dma_start(
                        out=out_d[pr, :, sb * SBLK : (sb + 1) * SBLK], in_=ot
                    )

    nc.compile()
    return nc


def _get_nc():
    with _lock:
        if _cache["nc"] is None:
            _cache["nc"] = _build()
        return _cache["nc"]


def _prep_inputs(Q, K, V):
    Q = np.asarray(Q, dtype=np.float32)
    K = np.asarray(K, dtype=np.float32)
    V = np.asarray(V, dtype=np.float32)
    # Q^T per head: [B, H, D, S], bf16 for full-rate PE streaming
    qt = np.ascontiguousarray(Q.transpose(0, 1, 3, 2)).astype(np.float16)
    kb = K.astype(np.float16)
    # V pre-swizzled to SBUF layout [B, H, p, t_tile, d], bf16
    vp = np.ascontiguousarray(
        V.reshape(B, H, NTT, P, D).transpose(0, 1, 3, 2, 4)
    ).astype(np.float16)
    in_maps = []
    for c in range(N_CORES):
        b, sc = divmod(c, S_CHUNKS)
        in_maps.append(
            {
                "QT": np.ascontiguousarray(
                    qt[b, :, :, sc * S_LOC : (sc + 1) * S_LOC]
                ),
                "K": np.ascontiguousarray(kb[b]),
                "V": vp[b],
            }
        )
    return in_maps


def _assemble(results):
    # The reference output is a RAW reshape of contiguous [B, H, S, d] to
    # [B, S, H*d] (torch .view quirk), NOT a head-transpose. So build
    # o[B, H, S, d] and reshape.
    o_full = np.empty((B, H, S, D), dtype=np.float32)
    for c in range(N_CORES):
        b, sc = divmod(c, S_CHUNKS)
        shard = results[c]["OUT"]  # [NPAIR, 128, S_LOC] = [pair, (half d), s]
        o_full[b, :, sc * S_LOC : (sc + 1) * S_LOC, :] = (
            shard.reshape(NPAIR, 2, D, S_LOC).transpose(0, 1, 3, 2).reshape(
                H, S_LOC, D
            )
        )
    return o_full.reshape(B, S, HIDDEN)


def run(Q, K, V, trace=False, **run_kwargs):
    nc = _get_nc()
    in_maps = _prep_inputs(Q, K, V)
    res = run_bass_kernel_spmd(
        nc, in_maps, core_ids=list(range(N_CORES)), trace=trace, **run_kwargs
    )
    return _assemble(res.results), res


def kernel(Q, K, V):
    # Force the no-trace path: the NTFF profile hook is not wired up in a
    # bare environment, and BASS_TRACE in the ambient env would crash.
    prev = os.environ.get("BASS_NEVER_TRACE")
    os.environ["BASS_NEVER_TRACE"] = "1"
    try:
        out, _ = run(Q, K, V, trace=False)
    finally:
        if prev is None:
            os.environ.pop("BASS_NEVER_TRACE", None)
        else:
            os.environ["BASS_NEVER_TRACE"] = prev
    return out
